# revision 16
# baseline (speedup 1.0000x reference)
"""Trainium2 Bass kernel for nn_DensePoseV1ConvXGNSparseHead.

8 layers of submanifold 3x3 conv (gather-GEMM over 9 taps) + GroupNorm(32)
+ ReLU on N=131072 sparse sites, 256->512 then 512->512 channels.

Strategy
--------
The 9-tap rulebook is a 3x3 stencil on a ~60%-occupied grid.  On the host we
reconstruct a planar embedding of the points from nbr_idx (min-label
propagation over the neighbor graph), pack the connected components into a
padded dense canvas, and run the conv as *dense* channel-major fp32r matmuls
with shifted access patterns: zero gather, zero transposes, contiguous DMA.
Inactive/pad cells are kept at exactly 0 by folding an activity mask into
the GroupNorm affine application, so submanifold semantics are preserved.

Sharding: canvas rows are split across the 8 cores with an 8-row halo on
each side - the full receptive field of 8 stacked 3x3 convs - so every core
computes its slice for all 8 layers with ZERO inter-core communication.
Conv weights / GN params are replicated (sharding_hint's halo all-gather is
avoided entirely by recomputing the halo locally).

Per layer, per 2048-column block, per 128-channel output chunk:
  - conv: 4ci x 9tap x 4subtile accumulating fp32r matmuls into PSUM
  - GroupNorm stats as PE matmuls with 0/1 group masks (partition reduce)
  - rsqrt via DVE reciprocal + ACT sqrt; normalize folded into two
    PE-broadcast matmuls (A = gamma*inv*mask, B = beta*mask - gamma*mu*inv*mask)
  - apply y*A+B on DVE, ReLU on ACT, write back fp32r.
"""

import numpy as np

import concourse.bass as bass
import concourse.tile as tile
from concourse import bacc, mybir
from concourse.bass_utils import run_bass_kernel_spmd

DT = mybir.dt
F32R = DT.float32r

N_TAPS = 9
OFFS = [(dy, dx) for dy in (-1, 0, 1) for dx in (-1, 0, 1)]
OFFS_ARR = np.array(OFFS, np.int64)
HALO_ROWS = 8
N_CORES = 8
BLOCK = 1536
SUB = 512  # psum subtile (fp32 bank)
NSUB = BLOCK // SUB
HA = 1024  # first conv half (psum tag ca, 2 banks); second half is SUB
HID = 512
GROUPS_PER_CHUNK = 8  # 32 groups / 4 chunks
GSIZE = 16
EPS = 1e-5
CPAD = 128  # zero columns left/right of the compute region (conv reads +-67)
WIN = BLOCK + 2 * 67  # x window per block


# ----------------------------------------------------------------- host side

def _embed_points(nbr):
    n = nbr.shape[0]
    assert nbr.shape[1] == N_TAPS
    assert (nbr[:, 4] == np.arange(n)).all(), "tap 4 must be self"
    comp = np.arange(n, dtype=np.int64)
    py = np.zeros(n, np.int64)
    px = np.zeros(n, np.int64)
    edges = []
    for k in range(N_TAPS):
        if k == 4:
            continue
        t = nbr[:, k]
        src = np.flatnonzero(t >= 0)
        edges.append((src, t[src].astype(np.int64), int(OFFS_ARR[k, 0]),
                      int(OFFS_ARR[k, 1])))
    for _ in range(100_000):
        changed = False
        for src, dst, dy, dx in edges:
            bad = comp[src] < comp[dst]
            if bad.any():
                s, d = src[bad], dst[bad]
                order = np.argsort(comp[s], kind="stable")
                s, d = s[order], d[order]
                uniq, first = np.unique(d, return_index=True)
                s, d = s[first], uniq
                comp[d] = comp[s]
                py[d] = py[s] + dy
                px[d] = px[s] + dx
                changed = True
        if not changed:
            break
    else:
        raise RuntimeError("label propagation did not converge")
    for k in range(N_TAPS):
        t = nbr[:, k]
        src = np.flatnonzero(t >= 0)
        dst = t[src]
        ok = ((comp[src] == comp[dst])
              & (py[dst] == py[src] + OFFS_ARR[k, 0])
              & (px[dst] == px[src] + OFFS_ARR[k, 1]))
        if not ok.all():
            raise RuntimeError(f"rulebook inconsistent at tap {k}")
    return comp, py, px


def _build_canvas_map(nbr):
    n = nbr.shape[0]
    comp, py, px = _embed_points(nbr)
    uniq, inv = np.unique(comp, return_inverse=True)
    ncmp = uniq.size
    big = 1 << 60
    miny = np.full(ncmp, big); minx = np.full(ncmp, big)
    maxy = np.full(ncmp, -big); maxx = np.full(ncmp, -big)
    np.minimum.at(miny, inv, py); np.minimum.at(minx, inv, px)
    np.maximum.at(maxy, inv, py); np.maximum.at(maxx, inv, px)
    h = maxy - miny + 1
    w = maxx - minx + 1
    stride = int(w.max()) + 2
    shelf_w = stride - 2

    # Pack components: big ones stacked vertically (full rows); small ones
    # shelf-packed side by side to avoid burning a full canvas row each.
    npts = np.bincount(inv)
    isbig = npts > 1000
    row_off = np.zeros(ncmp, np.int64)
    col_off = np.ones(ncmp, np.int64)
    acc = 0
    for c in np.flatnonzero(isbig):
        row_off[c] = acc
        acc += int(h[c]) + 1
    order = sorted(np.flatnonzero(~isbig), key=lambda c: -int(h[c]))
    shelf_row, shelf_h, xcur = acc, 0, 0
    for c in order:
        if xcur + int(w[c]) > shelf_w:
            shelf_row += shelf_h + 1
            shelf_h, xcur = 0, 0
        if shelf_h == 0:
            shelf_h = int(h[c])
        row_off[c] = shelf_row
        col_off[c] = 1 + xcur
        xcur += int(w[c]) + 1
    if xcur > 0:
        shelf_row += shelf_h + 1
    total_rows = int(shelf_row)
    r8 = -(-total_rows // N_CORES)
    rg = N_CORES * r8 + 2 * HALO_ROWS
    grow = HALO_ROWS + row_off[inv] + (py - miny[inv])
    gcol = col_off[inv] + (px - minx[inv])
    pos = grow * stride + gcol
    occupied = np.zeros(rg * stride, bool)
    if pos.max() >= occupied.size or np.unique(pos).size != n:
        raise RuntimeError("canvas build failed")
    occupied[pos] = True
    for k in range(N_TAPS):
        if k == 4:
            continue
        dpos = int(OFFS_ARR[k, 0]) * stride + int(OFFS_ARR[k, 1])
        if occupied[pos[nbr[:, k] < 0] + dpos].any():
            raise RuntimeError(f"tap {k}: active cell where rulebook says -1")
    m_raw = (r8 + 2 * HALO_ROWS) * stride
    m_pad = -(-m_raw // BLOCK) * BLOCK
    return pos, dict(stride=stride, r8=r8, rg=rg, m_raw=m_raw, m_pad=m_pad,
                     n_blocks=m_pad // BLOCK)


# --------------------------------------------------------------- bass program

def _build_program(m_pad, n_blocks, layers, stride):
    # extra tail slack so the cross-block prefetch may harmlessly overread
    padw = CPAD + m_pad + CPAD + BLOCK + 128
    nc = bacc.Bacc("TRN2", target_bir_lowering=False, debug=False)

    x0_d = nc.dram_tensor("x0", (2, 128, padw), F32R, kind="ExternalInput")
    w0_d = nc.dram_tensor("w0p", (128, N_TAPS * 2 * HID), F32R,
                          kind="ExternalInput")
    wr_d = nc.dram_tensor("wrp", (max(layers - 1, 1), 128, N_TAPS * 4 * HID),
                          F32R, kind="ExternalInput")
    acg_d = nc.dram_tensor("acg", (layers, 32, 2048), F32R, kind="ExternalInput")
    gm_d = nc.dram_tensor("gm16", (layers, 32, 2048), F32R, kind="ExternalInput")
    bc_d = nc.dram_tensor("bc32", (layers, 32, 2048), F32R, kind="ExternalInput")
    smask_d = nc.dram_tensor("smask", (128, 128), F32R, kind="ExternalInput")
    msk32_d = nc.dram_tensor("msk32", (32, m_pad // 3 + SUB), F32R,
                             kind="ExternalInput")
    out_d = nc.dram_tensor("out", (4, 128, m_pad), DT.float32,
                           kind="ExternalOutput")
    xa_d = nc.dram_tensor("xa", (4, 128, padw), F32R, kind="Internal")
    xb_d = nc.dram_tensor("xb", (4, 128, padw), F32R, kind="Internal")

    deltas = [dy * stride + dx for dy, dx in OFFS]

    with tile.TileContext(nc) as tc:
        with (
            tc.tile_pool(name="consts", bufs=1) as constp,
            tc.tile_pool(name="wp", bufs=1) as wpool,
            tc.tile_pool(name="lyc", bufs=2) as lycp,
            tc.tile_pool(name="xwp", bufs=1) as xwpool,
            tc.tile_pool(name="yb", bufs=3) as ypool,
            tc.tile_pool(name="ysq", bufs=2) as ysqpool,
            tc.tile_pool(name="tt", bufs=1) as ttpool,
            tc.tile_pool(name="tt2", bufs=2) as tt2pool,
            tc.tile_pool(name="psC", bufs=1, space=bass.MemorySpace.PSUM) as psCp,
            tc.tile_pool(name="psS", bufs=1, space=bass.MemorySpace.PSUM) as psSp,
            tc.tile_pool(name="psA", bufs=1, space=bass.MemorySpace.PSUM) as psAp,
            tc.tile_pool(name="psB", bufs=2, space=bass.MemorySpace.PSUM) as psBp,
        ):
            smask = constp.tile([128, 128], F32R)
            nc.sync.dma_start(smask[:], smask_d.ap())
            xw0 = constp.tile([128, 4 * WIN], F32R, tag="xw0")
            xw1 = constp.tile([128, 4 * WIN], F32R, tag="xw1")
            msk0 = constp.tile([32, SUB], F32R, tag="msk0")
            msk1 = constp.tile([32, SUB], F32R, tag="msk1")
            xwt = [xw0, xw1]
            mskt = [msk0, msk1]

            # zero the conv pads of the internal ping-pong buffers once
            zpad = constp.tile([128, CPAD], DT.float32)
            nc.gpsimd.memset(zpad[:], 0.0)
            for buf in (xa_d, xb_d):
                for ci in range(4):
                    nc.sync.dma_start(buf.ap()[ci, :, 0:CPAD],
                                      zpad[:].bitcast(F32R))
                    for z0 in range(CPAD + m_pad, padw, CPAD):
                        zw = min(CPAD, padw - z0)
                        nc.sync.dma_start(buf.ap()[ci, :, z0:z0 + zw],
                                          zpad[:, 0:zw].bitcast(F32R))

            def load_xw(p, src_aps, nci, bexpr):
                for ci in range(nci):
                    nc.sync.dma_start(
                        xwt[p][:, ci * WIN:(ci + 1) * WIN],
                        src_aps[ci][:, bass.ds(bexpr * BLOCK + (CPAD - 67),
                                               WIN)])
                nc.sync.dma_start(
                    mskt[p][:], msk32_d.ap()[:, bass.ds(bexpr * SUB, SUB)])

            def run_layer(li, nci, src_aps, wsrc_ap, dst_aps, final):
                w_sb = wpool.tile([128, N_TAPS * 4 * HID], F32R, tag="w")
                wq = N_TAPS * nci * 128  # cols per co chunk
                for co in range(4):
                    nc.sync.dma_start(w_sb[:, co * wq:(co + 1) * wq],
                                      wsrc_ap[:, co * wq:(co + 1) * wq])

                def conv_half(co, j0, j1, ps):
                    nmm = nci * N_TAPS
                    mi = 0
                    for ci in range(nci):
                        for k in range(N_TAPS):
                            woff = (co * nci * N_TAPS + k * nci + ci) * 128
                            lhsT = w_sb[:, woff:woff + 128]
                            base = ci * WIN + 67 + deltas[k]
                            for j in range(j0, j1):
                                nc.tensor.matmul(
                                    ps[:, (j - j0) * SUB:(j - j0 + 1) * SUB],
                                    lhsT,
                                    xwt[cur][:, base + j * SUB:
                                              base + j * SUB + SUB],
                                    start=(mi == 0), stop=(mi == nmm - 1))
                            mi += 1

                def ep_stats(co, y, ysq, bexpr):
                    msk = mskt[cur]
                    psX = psSp.tile([32, SUB], DT.float32, tag="sx")
                    psXX = psSp.tile([32, SUB], DT.float32, tag="sxx")
                    acg = lycp.tile([32, SUB], F32R, tag="acg")
                    nc.sync.dma_start(acg[:],
                                      acg_d.ap()[li, :, co * SUB:(co + 1) * SUB])
                    gm = lycp.tile([32, SUB], F32R, tag="gm")
                    nc.sync.dma_start(gm[:],
                                      gm_d.ap()[li, :, co * SUB:(co + 1) * SUB])
                    bc = lycp.tile([32, SUB], F32R, tag="bc")
                    nc.sync.dma_start(bc[:],
                                      bc_d.ap()[li, :, co * SUB:(co + 1) * SUB])
                    for j in range(NSUB):
                        nc.tensor.matmul(psX[:],
                                         smask[:, j * 32:(j + 1) * 32],
                                         y[:, j * SUB:(j + 1) * SUB],
                                         start=(j == 0), stop=(j == NSUB - 1))
                    for j in range(NSUB):
                        nc.tensor.matmul(psXX[:],
                                         smask[:, j * 32:(j + 1) * 32],
                                         ysq[:, j * SUB:(j + 1) * SUB],
                                         start=(j == 0), stop=(j == NSUB - 1))

                    sxs = ttpool.tile([32, SUB], F32R, tag="sxs")
                    nc.vector.tensor_copy(sxs[:], psX[:])
                    u = ttpool.tile([32, SUB], DT.float32, tag="u")
                    nc.vector.tensor_tensor(u[:], sxs[:], sxs[:],
                                            mybir.AluOpType.mult)
                    u2 = ttpool.tile([32, SUB], DT.float32, tag="u2")
                    nc.vector.tensor_scalar(u2[:], u[:], -1.0 / GSIZE, None,
                                            mybir.AluOpType.mult)
                    v = ttpool.tile([32, SUB], DT.float32, tag="v")
                    nc.vector.tensor_tensor(v[:], psXX[:], u2[:],
                                            mybir.AluOpType.add)
                    uu = ttpool.tile([32, SUB], DT.float32, tag="u")
                    nc.vector.tensor_scalar(uu[:], v[:], 1.0 / GSIZE, EPS,
                                            mybir.AluOpType.mult,
                                            mybir.AluOpType.add)
                    r = ttpool.tile([32, SUB], DT.float32, tag="r")
                    nc.vector.reciprocal_approx_fast(r[:], uu[:])
                    inv = ttpool.tile([32, SUB], DT.float32, tag="u2")
                    nc.scalar.activation(inv[:], r[:],
                                         mybir.ActivationFunctionType.Sqrt)
                    invm = tt2pool.tile([32, SUB], F32R, tag="invm")
                    nc.vector.tensor_tensor(invm[:], inv[:], msk[:],
                                            mybir.AluOpType.mult)
                    w32 = tt2pool.tile([32, SUB], F32R, tag="w32")
                    nc.vector.tensor_tensor(w32[:], sxs[:], invm[:],
                                            mybir.AluOpType.mult)
                    return invm, w32, msk, acg, gm, bc

                def ep_ab(co, y, invm, w32, msk, acg, gm, bc, bexpr):
                    for j in range(NSUB):
                        cj = j * 128
                        psA = psAp.tile([128, SUB], DT.float32, tag="A")
                        nc.tensor.matmul(psA[:], acg[:, cj:cj + 128],
                                         invm[:], start=True, stop=True)
                        psB = psBp.tile([128, SUB], DT.float32, tag="B")
                        nc.tensor.matmul(psB[:], bc[:, cj:cj + 128],
                                         msk[:], start=True, stop=False)
                        nc.tensor.matmul(psB[:], gm[:, cj:cj + 128],
                                         w32[:], start=False, stop=True)
                        t1 = tt2pool.tile([128, SUB], DT.float32, tag="t1")
                        nc.vector.tensor_tensor(
                            t1[:], psA[:], y[:, j * SUB:(j + 1) * SUB],
                            mybir.AluOpType.mult)
                        t2 = tt2pool.tile([128, SUB], DT.float32, tag="t2")
                        nc.vector.tensor_tensor(t2[:], psB[:], t1[:],
                                                mybir.AluOpType.add)
                        nc.scalar.activation(
                            y[:, j * SUB:(j + 1) * SUB], t2[:],
                            mybir.ActivationFunctionType.Relu)

                    dst = dst_aps[co][:, bass.ds(bexpr * BLOCK + (0 if final
                                                                  else CPAD),
                                                 BLOCK)]
                    nc.sync.dma_start(dst, y[:] if not final
                                      else y[:].bitcast(DT.float32))

                def run_block(bexpr, pre_bexpr):
                    # prefetch next block's window while this one computes
                    load_xw(1 - cur, src_aps, nci, pre_bexpr)
                    pstat = []
                    pab = []
                    for co in range(4):
                        y = ypool.tile([128, BLOCK], F32R, tag="y")
                        ysq = ysqpool.tile([128, BLOCK], F32R, tag="ysq")
                        psa = psCp.tile([128, HA], DT.float32, tag="ca")
                        conv_half(co, 0, 2, psa)
                        nc.vector.tensor_copy(y[:, 0:HA], psa[:])
                        nc.vector.tensor_tensor(
                            ysq[:, 0:HA], y[:, 0:HA], y[:, 0:HA],
                            mybir.AluOpType.mult)
                        psb = psCp.tile([128, SUB], DT.float32, tag="cb")
                        conv_half(co, 2, NSUB, psb)
                        nc.vector.tensor_copy(y[:, HA:BLOCK], psb[:])
                        nc.vector.tensor_tensor(
                            ysq[:, HA:BLOCK], y[:, HA:BLOCK], y[:, HA:BLOCK],
                            mybir.AluOpType.mult)
                        pstat.append((co, y, ysq))
                        if len(pstat) > 1:
                            c_, y_, ysq_ = pstat.pop(0)
                            st = ep_stats(c_, y_, ysq_, bexpr)
                            pab.append((c_, y_) + st)
                        if len(pab) > 1:
                            ep_ab(*pab.pop(0), bexpr)
                    while pstat:
                        c_, y_, ysq_ = pstat.pop(0)
                        st = ep_stats(c_, y_, ysq_, bexpr)
                        pab.append((c_, y_) + st)
                        while len(pab) > 1:
                            ep_ab(*pab.pop(0), bexpr)
                    while pab:
                        ep_ab(*pab.pop(0), bexpr)

                # prologue: window for block 0
                cur = 0
                load_xw(0, src_aps, nci, 0)
                nbe = n_blocks - (n_blocks % 2)
                if nbe:
                    with tc.For_i(0, nbe, 2,
                                  hint_engines=(mybir.EngineType.PE,)) as i:
                        run_block(i, i + 1)
                        cur = 1
                        run_block(i + 1, i + 2)
                        cur = 0
                if n_blocks % 2:
                    run_block(nbe, nbe)
                tc.strict_bb_all_engine_barrier()

            bufs = {"x0": x0_d, "xa": xa_d, "xb": xb_d}
            seq = ["x0"] + ["xa", "xb"] * 4
            for li in range(layers):
                src, dst = seq[li], seq[li + 1]
                nci = 2 if li == 0 else 4
                src_aps = [bufs[src].ap()[ci] for ci in range(nci)]
                final = li == layers - 1
                dst_aps = ([out_d.ap()[co] for co in range(4)] if final
                           else [bufs[dst].ap()[co] for co in range(4)])
                wsrc = (w0_d.ap() if li == 0
                        else wr_d.ap()[li - 1, :, 0:N_TAPS * 4 * HID])
                run_layer(li, nci, src_aps, wsrc, dst_aps, final)

    nc.compile()
    return nc


# ------------------------------------------------------------- host packing

def _pack_host(inputs, pos, meta, layers=8):
    feats = np.ascontiguousarray(np.asarray(inputs["features"], np.float32))
    w0 = np.asarray(inputs["w0"], np.float32)
    w_rest = np.asarray(inputs["w_rest"], np.float32)
    gamma = np.asarray(inputs["gamma"], np.float32)
    beta = np.asarray(inputs["beta"], np.float32)
    n, cin = feats.shape
    stride, r8, m_pad = meta["stride"], meta["r8"], meta["m_pad"]
    rgst = meta["rg"] * stride
    padw = CPAD + m_pad + CPAD + BLOCK + 128

    x_g = np.zeros((cin, rgst), np.float32)
    x_g[:, pos] = feats.T
    mask_g = np.zeros(rgst, np.float32)
    mask_g[pos] = 1.0

    # weights: wpk[p, co, k, ci, :] = w[k, ci*128+p, co*128:(co+1)*128]
    w0p = np.ascontiguousarray(
        w0.reshape(N_TAPS, 2, 128, 4, 128).transpose(2, 3, 0, 1, 4)
    ).reshape(128, N_TAPS * 2 * HID)
    nl = max(layers - 1, 1)
    wrp = np.ascontiguousarray(
        w_rest[:layers - 1].reshape(layers - 1, N_TAPS, 4, 128, 4, 128)
        .transpose(0, 3, 4, 1, 2, 5)
    ).reshape(layers - 1, 128, N_TAPS * 4 * HID)
    if wrp.shape[0] < nl:
        wrp = np.zeros((nl, 128, N_TAPS * 4 * HID), np.float32)

    ch = np.arange(128)
    acg = np.zeros((layers, 32, 4, 4, 128), np.float32)
    gm16 = np.zeros((layers, 32, 4, 4, 128), np.float32)
    bc32 = np.zeros((layers, 32, 4, 4, 128), np.float32)
    for li in range(layers):
        for co in range(4):
            g_ = gamma[li, co * 128:(co + 1) * 128]
            b_ = beta[li, co * 128:(co + 1) * 128]
            for j in range(4):
                rows = 8 * j + ch // GSIZE
                acg[li, rows, co, j, ch] = g_
                gm16[li, rows, co, j, ch] = -g_ / GSIZE
                bc32[li, 8 * j, co, j, :] = b_
    acg = acg.reshape(layers, 32, 2048)
    gm16 = gm16.reshape(layers, 32, 2048)
    bc32 = bc32.reshape(layers, 32, 2048)

    smask = np.zeros((128, 4, 32), np.float32)
    for j in range(4):
        smask[ch, j, 8 * j + ch // GSIZE] = 1.0
    smask = smask.reshape(128, 128)

    in_maps = []
    for s in range(N_CORES):
        c0 = s * r8 * stride
        x0 = np.zeros((2, 128, padw), np.float32)
        seg = x_g[:, c0:min(c0 + m_pad, rgst)]
        x0[:, :, CPAD:CPAD + seg.shape[1]] = seg.reshape(2, 128, -1)
        mc = np.zeros(m_pad, np.float32)
        mseg = mask_g[c0:min(c0 + m_pad, rgst)]
        mc[:mseg.shape[0]] = mseg
        # msk32[8j+g, b*512+c] = mask[b*2048 + j*512 + c]
        m4 = mc.reshape(-1, NSUB, SUB)  # [nb, j, c]
        msk32 = np.zeros((32, m_pad // NSUB + SUB), np.float32)
        for j in range(NSUB):
            for g in range(8):
                msk32[8 * j + g, :m_pad // NSUB] = m4[:, j, :].reshape(-1)
        in_maps.append({
            "x0": x0, "w0p": w0p, "wrp": wrp, "acg": acg, "gm16": gm16,
            "bc32": bc32, "smask": smask, "msk32": msk32,
        })
    return in_maps


TRACE = False
LAST_RESULT = {}


def kernel(**inputs) -> np.ndarray:
    nbr = np.asarray(inputs["nbr_idx"])
    n = nbr.shape[0]
    pos, meta = _build_canvas_map(nbr)
    in_maps = _pack_host(inputs, pos, meta)
    nc = _build_program(meta["m_pad"], meta["n_blocks"], 8, meta["stride"])
    res = run_bass_kernel_spmd(nc, in_maps, list(range(N_CORES)), trace=TRACE)
    LAST_RESULT["exec_time_ns"] = res.exec_time_ns
    LAST_RESULT["profile_json"] = res.profile_json

    stride, r8 = meta["stride"], meta["r8"]
    row = pos // stride
    own = np.clip((row - HALO_ROWS) // r8, 0, N_CORES - 1)
    result = np.zeros((n, HID), np.float32)
    for s in range(N_CORES):
        sel = own == s
        local = pos[sel] - s * r8 * stride
        o = res.results[s]["out"]  # [4, 128, m_pad]
        result[sel] = o[:, :, local].reshape(HID, -1).T
    return result


if __name__ == "__main__":
    import reference

    inputs = reference.setup_inputs()
    out = kernel(**{k: np.asarray(v) for k, v in inputs.items()})
    exp = np.asarray(reference.reference(**inputs))
    err = np.linalg.norm(out - exp) / np.linalg.norm(exp)
    print(f"l2 rel err: {err:.3e}")
